# revision 27
# baseline (speedup 1.0000x reference)
"""Trainium2 Bass kernel for nn_GCN_23029614641773.

The reference GCN operates on B independent 27-node graphs where every node of
graph i starts with the same feature vector x[i], and only node 0 of each graph
feeds the classifier head. Exploiting linearity of the edge aggregation, the
whole network collapses exactly (up to fp rounding order) to a per-sample MLP:

    y = x @ W0                                  # [B, 1024]
    s = lrelu(y + b0) + 2*lrelu(3y + b0) + lrelu(5y + b0)
      # node 1's in-neighbours {0,2,4,6} have in-degrees {1,3,3,5}.
      # With b0 == 0 (spec fill): s == 12*lrelu(y); the 12 is folded into W1
      # host-side (fp16 rounding of 12*W1 == rounding of W1, same rel err).
    t = s @ W1;  h = lrelu(t + b1)              # [B, 512]
    v = h @ W2;  g = lrelu(v + b2)              # [B, 256]
    out = g @ Wc + bc                           # [B, 1]

Sharding: pure data parallelism, batch split across 8 NeuronCores; each core
holds the full weight set.

Perf design (from ntff trace analysis; measured facts in parentheses):
- PE cadence is optimal when fed: warm fp16 MMs issue every N/2.4GHz+2.5ns
  with LDWEIGHTS pulled ahead and hidden (~109ns/MM at N=256); the PE
  floor for the 106 real MMs is ~11.6us.
- The kernel is DMA-bound on its edges: 3.93MB of fp16 inputs at ~420GB/s
  aggregate (16 SDMA engines x 26.5GB/s). The old single trailing
  W1+W2+Wc DMA completed ~2.4us after L1 drained, stalling the PE; that
  idle tripped the PE_HAM MID window and re-throttled the PE clock to
  1.2GHz for 3.4us. Fix: bulk loads split into 0.25-0.75MB chunks in
  strict consumption order, margins growing toward the tail (>=2us) so a
  single slow SDMA engine (observed hiccups: 0.7-1.5us with head-of-line
  blocking on that engine's FIFO) cannot stall the PE.
- Chunk rows stay >=4KB: 2KB-row streams measured ~40% lower aggregate
  rate. The 2KB-row W2+Wc tail rides last with ~4us of slack.
- x^T + W0 m0-m1 ride in ONE leading 1MB DMA: the first ~0.3MB of the
  stream transfers at ramp-up rate regardless (a tiny pre-warm DMA was
  measured a net loss), and a smaller leading chunk just moves the stall
  to W0 m1 on slow-DMA runs.
- PE warmup: 19 dummy matmuls (~4us at the 1.2GHz cold clock) flip the
  HAM clock gate (needs ~3.4us sustained activity) and end just before
  the earliest plausible x receipt, so the warmup never delays real work.
- All activations are plain lrelu (the x12 degree factor is folded into
  W1 host-side). The last tile of each layer gets a dedicated engine so
  it never queues: second-to-last on the DVE (mul+max pair; a single
  scalar_tensor_tensor cannot read PSUM twice), last on the then-idle
  Scalar ACT.
- L3's two m-blocks accumulate k-interleaved so both psums complete right
  after h3's activation; g0 (first-complete) takes the single-instruction
  Scalar ACT, g1 overlaps on the DVE; classifier matmuls chase them.
"""

from contextlib import ExitStack

import numpy as np

import concourse.bacc as bacc
import concourse.mybir as mybir
import concourse.tile as tile
from concourse.bass_utils import run_bass_kernel_spmd

F32 = mybir.dt.float32
F16 = mybir.dt.float16
P = 128
N_CORES = 8
B_FULL = 2048
B = B_FULL // N_CORES  # 256 rows per core
D0, D1, D2, D3 = 1024, 1024, 512, 256
K0, M0 = D0 // P, D1 // P  # 8, 8
K1, M1 = D1 // P, D2 // P  # 8, 4
K2, M2 = D2 // P, D3 // P  # 4, 2
KC = D3 // P  # 2

NEG_SLOPE = 0.2

# --- tunables (A/B'd on hardware) ---
# 19 x 213ns = 4.0us of sustained cold-clock activity: enough to flip the
# HAM clock gate (needs ~3.4us), ending just before the earliest plausible
# x-DMA receipt so the warmup never delays the first real matmul.
NWARM = 19

def _build(zero_bias: bool):
    nc = bacc.Bacc(
        "TRN2", target_bir_lowering=False, debug=False,
        enable_asserts=False, num_devices=1,
    )

    # Bulk inputs, split into chunks ordered by consumption time. Chunk k's
    # completion receipt must land before its first consumer; margins grow
    # toward the tail so SDMA hiccups can't stall the PE. DMA element size
    # (row bytes) stays >= 4KB — 2KB-row streams measured ~40% slower
    # aggregate. The W2+Wc tail (2KB rows) rides last with ~4us of slack.
    xw_d = nc.dram_tensor("xw", [P, K0 * B + 2 * K0 * P], F16,
                          kind="ExternalInput").ap()        # x^T + W0 m0-m1
    w0m23_d = nc.dram_tensor("w0m23", [P, 2 * K0 * P], F16,
                             kind="ExternalInput").ap()
    w0m45_d = nc.dram_tensor("w0m45", [P, 2 * K0 * P], F16,
                             kind="ExternalInput").ap()
    w0m67w1m0_d = nc.dram_tensor("w0m67w1m0", [P, 2 * K0 * P + K1 * P], F16,
                                 kind="ExternalInput").ap()
    w1m123_d = nc.dram_tensor("w1m123", [P, 3 * K1 * P], F16,
                              kind="ExternalInput").ap()
    w2wc_d = nc.dram_tensor("w2wc", [P, M2 * K2 * P + KC], F16,
                            kind="ExternalInput").ap()
    if not zero_bias:
        b0_d = nc.dram_tensor("b0", [D1], F32, kind="ExternalInput").ap()
        b1_d = nc.dram_tensor("b1", [D2], F32, kind="ExternalInput").ap()
        b2_d = nc.dram_tensor("b2", [D3], F32, kind="ExternalInput").ap()
        bc_d = nc.dram_tensor("bc", [1], F32, kind="ExternalInput").ap()
    out_d = nc.dram_tensor("out", [1, B], F32, kind="ExternalOutput").ap()

    with ExitStack() as ctx:
        tc = ctx.enter_context(tile.TileContext(nc))
        const = ctx.enter_context(tc.tile_pool(name="const", bufs=1))
        xt_p = ctx.enter_context(tc.tile_pool(name="xt", bufs=1))
        w_p = ctx.enter_context(tc.tile_pool(name="w", bufs=8))
        s_p = ctx.enter_context(tc.tile_pool(name="s", bufs=K1))
        h_p = ctx.enter_context(tc.tile_pool(name="h", bufs=K2))
        g_p = ctx.enter_context(tc.tile_pool(name="g", bufs=KC))
        tmp_p = ctx.enter_context(tc.tile_pool(name="tmp", bufs=4))
        out_p = ctx.enter_context(tc.tile_pool(name="outp", bufs=1))
        ps_p = ctx.enter_context(tc.tile_pool(name="ps", bufs=6, space="PSUM"))
        cls_ps = ctx.enter_context(tc.tile_pool(name="cls", bufs=1, space="PSUM"))
        warm_ps = ctx.enter_context(tc.tile_pool(name="warm", bufs=1,
                                                 space="PSUM"))

        # leaky-relu slope as a per-partition alpha vector for ACT Prelu
        alt = const.tile([P, 1], F32, tag="alt")
        nc.vector.memset(alt[:], NEG_SLOPE)

        # ---- PE warmup: HAM clock gate defaults to 4/8 (1.2GHz); ~3.4us of
        # sustained activity flips it to 8/8. Fill the input-DMA wait with a
        # zero matmul accumulation group sized to end ~when the first input
        # DMA's receipt fires; it must run CONTIGUOUSLY into the real work. ----
        wz = const.tile([P, B], F16, tag="wz")
        nc.vector.memset(wz[:], 0.0)
        pw = warm_ps.tile([P, B], F32)
        for i in range(NWARM):
            nc.tensor.matmul(pw[:], lhsT=wz[:, 0:P], rhs=wz[:],
                             start=(i == 0), stop=(i == NWARM - 1))

        # ---- DMA plan: ONE queue (sync HWDGE), chunks in consumption order.
        # HWDGE descriptor generation costs ~650ns per dma_start on the
        # issuing sequencer; 7 DMAs ~ 4.6us, which stays ahead of the
        # transfer stream. ----
        xw_t = xt_p.tile([P, K0 * B + 2 * K0 * P], F16, tag="xt", name="xw")
        nc.sync.dma_start(xw_t[:], xw_d)
        xt = [xw_t[:, k * B:(k + 1) * B] for k in range(K0)]

        chunks = []
        for nm, dram, cols in (
            ("w0m23", w0m23_d, 2 * K0 * P),
            ("w0m45", w0m45_d, 2 * K0 * P),
            ("w0m67w1m0", w0m67w1m0_d, 2 * K0 * P + K1 * P),
            ("w1m123", w1m123_d, 3 * K1 * P),
            ("w2wc", w2wc_d, M2 * K2 * P + KC),
        ):
            t = w_p.tile([P, cols], F16, tag="w", name=nm)
            nc.sync.dma_start(t[:], dram)
            chunks.append(t)
        w0m23_t, w0m45_t, w0m67w1m0_t, w1m123_t, w2wc_t = chunks
        wc = w2wc_t[:, M2 * K2 * P:M2 * K2 * P + KC]

        def w0_lhsT(m, k):
            if m < 2:
                off = K0 * B + (m * K0 + k) * P
                return xw_t[:, off:off + P]
            if m in (2, 3):
                off = ((m - 2) * K0 + k) * P
                return w0m23_t[:, off:off + P]
            if m in (4, 5):
                off = ((m - 4) * K0 + k) * P
                return w0m45_t[:, off:off + P]
            off = ((m - 6) * K0 + k) * P
            return w0m67w1m0_t[:, off:off + P]

        def w1_lhsT(m, k):
            if m == 0:
                off = 2 * K0 * P + k * P
                return w0m67w1m0_t[:, off:off + P]
            off = ((m - 1) * K1 + k) * P
            return w1m123_t[:, off:off + P]

        def w2_lhsT(m, k):
            return w2wc_t[:, (m * K2 + k) * P:(m * K2 + k + 1) * P]

        if not zero_bias:
            b0t = const.tile([P, M0], F32, tag="b0t")
            nc.scalar.dma_start(b0t[:], b0_d.rearrange("(c p) -> p c", p=P))
            b1t = const.tile([P, M1], F32, tag="b1t")
            nc.scalar.dma_start(b1t[:], b1_d.rearrange("(c p) -> p c", p=P))
            b2t = const.tile([P, M2], F32, tag="b2t")
            nc.scalar.dma_start(b2t[:], b2_d.rearrange("(c p) -> p c", p=P))
            bct = const.tile([1, 1], F32, tag="bct")
            nc.scalar.dma_start(bct[:], bc_d.rearrange("(a b) -> a b", a=1))
            b0t2 = const.tile([P, M0], F32, tag="b0t2")
            nc.vector.tensor_scalar_mul(b0t2[:], b0t[:], 2.0)

        PRELU = mybir.ActivationFunctionType.Prelu

        def matmul_group(ps, lhsT_fn, m, rhs_tiles, K):
            for k in range(K):
                nc.tensor.matmul(
                    ps[:], lhsT=lhsT_fn(m, k),
                    rhs=rhs_tiles[k],
                    start=(k == 0), stop=(k == K - 1),
                )

        def lrelu_dve(dst, ps, name):
            # DVE pair: max(0.2*v, v). A single scalar_tensor_tensor with
            # in0==in1==psum is rejected (one PSUM read port), so stage
            # 0.2*v in SBUF first.
            t = tmp_p.tile([P, B], F32, tag="l", name=name)
            nc.vector.tensor_scalar_mul(t[:], ps, NEG_SLOPE)
            nc.vector.tensor_max(dst, ps, t[:])

        # ---- layer 1: y[m] = sum_k W0[k,m].T @ xT[k]; s = lrelu(y)
        # (the degree-sum factor 12 is pre-folded into W1). The last
        # m-block's act runs on the DVE so L2 isn't queued behind the
        # Scalar-engine ACT pipeline. ----
        s_tiles = []
        for m in range(M0):
            ps = ps_p.tile([P, B], F32, tag="ps", name=f"ps1_{m}")
            matmul_group(ps, w0_lhsT, m, xt, K0)
            s = s_p.tile([P, B], F16, tag="s", name=f"s_{m}")
            if zero_bias:
                # second-to-last act on the DVE so the Scalar queue is empty
                # when the last m-block's psum completes -> its ACT starts
                # immediately and L2 isn't held up.
                if m == M0 - 2:
                    lrelu_dve(s[:], ps[:], f"sv_{m}")
                else:
                    nc.scalar.activation(s[:], ps[:], PRELU, alpha=alt[:])
            else:
                acc = tmp_p.tile([P, B], F32, tag="l", name=f"acc_{m}")
                first = True
                for scale, bias in ((1.0, b0t[:, m:m + 1]), (6.0, b0t2[:, m:m + 1]),
                                    (5.0, b0t[:, m:m + 1])):
                    l = tmp_p.tile([P, B], F32, tag="l", name=f"l_{m}")
                    nc.scalar.activation(l[:], ps[:], PRELU,
                                         scale=scale, bias=bias, alpha=alt[:])
                    if first:
                        nc.vector.tensor_copy(acc[:], l[:])
                        first = False
                    else:
                        nc.vector.tensor_add(acc[:], acc[:], l[:])
                nc.vector.tensor_copy(s[:], acc[:])
            s_tiles.append(s)

        # ---- layer 2: t[m] = sum_k W1[k,m].T @ s[k]; h = lrelu(t + b1) ----
        h_tiles = []
        for m in range(M1):
            ps = ps_p.tile([P, B], F32, tag="ps", name=f"ps2_{m}")
            matmul_group(ps, w1_lhsT, m, [t[:] for t in s_tiles], K1)
            h = h_p.tile([P, B], F16, tag="h", name=f"h_{m}")
            if zero_bias:
                if m == M1 - 2:
                    lrelu_dve(h[:], ps[:], f"hv_{m}")
                else:
                    nc.scalar.activation(h[:], ps[:], PRELU, alpha=alt[:])
            else:
                nc.scalar.activation(h[:], ps[:], PRELU,
                                     bias=b1t[:, m:m + 1], alpha=alt[:])
            h_tiles.append(h)

        # ---- layer 3, k-interleaved across the two m-blocks so both psums
        # complete right after h3's activation; g0 act on DVE and g1 act on
        # Scalar run concurrently; classifier matmuls chase them. ----
        ps30 = ps_p.tile([P, B], F32, tag="ps", name="ps3_0")
        ps31 = ps_p.tile([P, B], F32, tag="ps", name="ps3_1")
        for k in range(K2):
            nc.tensor.matmul(ps30[:], lhsT=w2_lhsT(0, k), rhs=h_tiles[k][:],
                             start=(k == 0), stop=(k == K2 - 1))
            nc.tensor.matmul(ps31[:], lhsT=w2_lhsT(1, k), rhs=h_tiles[k][:],
                             start=(k == 0), stop=(k == K2 - 1))
        g0 = g_p.tile([P, B], F16, tag="g", name="g_0")
        g1 = g_p.tile([P, B], F16, tag="g", name="g_1")
        if zero_bias:
            # the slower DVE mul+max pair gets the earlier-completing psum
            # (ps30); the single-instruction Scalar ACT takes ps31.
            lrelu_dve(g0[:], ps30[:], "gv")
            nc.scalar.activation(g1[:], ps31[:], PRELU, alpha=alt[:])
        else:
            nc.scalar.activation(g0[:], ps30[:], PRELU,
                                 bias=b2t[:, 0:1], alpha=alt[:])
            nc.scalar.activation(g1[:], ps31[:], PRELU,
                                 bias=b2t[:, 1:2], alpha=alt[:])

        # accumulation order is free: start with g1 (ready first via the
        # Scalar ACT) so the last matmul chases only g0's DVE pair.
        po = cls_ps.tile([1, B], F32)
        nc.tensor.matmul(po[:], lhsT=wc[:, 1:2], rhs=g1[:],
                         start=True, stop=False)
        nc.tensor.matmul(po[:], lhsT=wc[:, 0:1], rhs=g0[:],
                         start=False, stop=True)

        ob = out_p.tile([1, B], F32)
        if zero_bias:
            nc.vector.tensor_copy(ob[:], po[:])
        else:
            nc.vector.tensor_scalar_add(ob[:], po[:], bct[:, 0:1])
        nc.sync.dma_start(out_d, ob[:])

    nc.compile()
    return nc


_CACHE = {}


def _get_nc(zero_bias: bool):
    if zero_bias not in _CACHE:
        _CACHE[zero_bias] = _build(zero_bias)
    return _CACHE[zero_bias]


def _run(inputs, trace=False, **kw):
    def f32(a):
        return np.ascontiguousarray(np.asarray(a), dtype=np.float32)

    x = f32(inputs["x"])
    W0, b0 = f32(inputs["W0"]), f32(inputs["b0"])
    W1, b1 = f32(inputs["W1"]), f32(inputs["b1"])
    W2, b2 = f32(inputs["W2"]), f32(inputs["b2"])
    Wc, bc = f32(inputs["Wc"]), f32(inputs["bc"])
    zero_bias = not (b0.any() or b1.any() or b2.any() or bc.any())
    nc = _get_nc(zero_bias)

    # Host-side packing into exact SBUF tile layouts, fp16.
    def pack_w(W, K, M):
        return np.ascontiguousarray(
            W.astype(np.float16).reshape(K, P, M, P)
            .transpose(1, 2, 0, 3).reshape(P, M * K * P))

    w0p = pack_w(W0, K0, M0)
    # degree-sum factor of node 1's in-neighbourhood, folded into W1
    w1p = pack_w(W1 * 12.0 if zero_bias else W1, K1, M1)
    w2p = pack_w(W2, K2, M2)
    wcp = Wc.astype(np.float16)[:, 0].reshape(KC, P).T  # [128, 2]

    KP = K0 * P
    common = {
        "w0m23": np.ascontiguousarray(w0p[:, 2 * KP:4 * KP]),
        "w0m45": np.ascontiguousarray(w0p[:, 4 * KP:6 * KP]),
        "w0m67w1m0": np.ascontiguousarray(
            np.concatenate([w0p[:, 6 * KP:8 * KP], w1p[:, 0:K1 * P]], axis=1)),
        "w1m123": np.ascontiguousarray(w1p[:, K1 * P:4 * K1 * P]),
        "w2wc": np.ascontiguousarray(np.concatenate([w2p, wcp], axis=1)),
    }

    in_maps = []
    for i in range(N_CORES):
        xs = x[i * B:(i + 1) * B].astype(np.float16)  # [256, 1024]
        xtp = xs.T.reshape(K0, P, B).transpose(1, 0, 2).reshape(P, K0 * B)
        xwp = np.ascontiguousarray(
            np.concatenate([xtp, w0p[:, 0:2 * KP]], axis=1))
        m = {"xw": xwp, **common}
        if not zero_bias:
            m.update({"b0": b0, "b1": b1, "b2": b2, "bc": bc})
        in_maps.append(m)
    res = run_bass_kernel_spmd(nc, in_maps, list(range(N_CORES)),
                               trace=trace, **kw)
    out = np.empty((B_FULL, 1), dtype=np.float32)
    for i in range(N_CORES):
        out[i * B:(i + 1) * B, 0] = res.results[i]["out"][0]
    return out, res


def kernel(**inputs) -> np.ndarray:
    out, _ = _run(inputs)
    return out


# revision 28
# speedup vs baseline: 1.0838x; 1.0838x over previous
"""Trainium2 Bass kernel for nn_GCN_23029614641773.

The reference GCN operates on B independent 27-node graphs where every node of
graph i starts with the same feature vector x[i], and only node 0 of each graph
feeds the classifier head. Exploiting linearity of the edge aggregation, the
whole network collapses exactly (up to fp rounding order) to a per-sample MLP:

    y = x @ W0                                  # [B, 1024]
    s = lrelu(y + b0) + 2*lrelu(3y + b0) + lrelu(5y + b0)
      # node 1's in-neighbours {0,2,4,6} have in-degrees {1,3,3,5}.
      # With b0 == 0 (spec fill): s == 12*lrelu(y); the 12 is folded into W1
      # host-side (fp16 rounding of 12*W1 == rounding of W1, same rel err).
    t = s @ W1;  h = lrelu(t + b1)              # [B, 512]
    v = h @ W2;  g = lrelu(v + b2)              # [B, 256]
    out = g @ Wc + bc                           # [B, 1]

Sharding: pure data parallelism, batch split across 8 NeuronCores; each core
holds the full weight set.

Perf design (from ntff trace analysis; measured facts in parentheses):
- PE cadence is optimal when fed: warm fp16 MMs issue every N/2.4GHz+2.5ns
  with LDWEIGHTS pulled ahead and hidden (~109ns/MM at N=256); the PE
  floor for the 106 real MMs is ~11.6us.
- The kernel is DMA-bound on its edges: 3.93MB of fp16 inputs at ~420GB/s
  aggregate (16 SDMA engines x 26.5GB/s). The old single trailing
  W1+W2+Wc DMA completed ~2.4us after L1 drained, stalling the PE; that
  idle tripped the PE_HAM MID window and re-throttled the PE clock to
  1.2GHz for 3.4us. Fix: bulk loads split into 0.25-0.75MB chunks in
  strict consumption order, margins growing toward the tail (>=2us) so a
  single slow SDMA engine (observed hiccups: 0.7-1.5us with head-of-line
  blocking on that engine's FIFO) cannot stall the PE.
- Chunk rows stay >=4KB: 2KB-row streams measured ~40% lower aggregate
  rate. The 2KB-row W2+Wc tail rides last with ~4us of slack.
- x^T + W0 m0-m1 ride in ONE leading 1MB DMA: the first ~0.3MB of the
  stream transfers at ramp-up rate regardless (a tiny pre-warm DMA was
  measured a net loss), and a smaller leading chunk just moves the stall
  to W0 m1 on slow-DMA runs.
- PE warmup: 19 dummy matmuls (~4us at the 1.2GHz cold clock) flip the
  HAM clock gate (needs ~3.4us sustained activity) and end just before
  the earliest plausible x receipt, so the warmup never delays real work.
- All activations are plain lrelu (the x12 degree factor is folded into
  W1 host-side). The last tile of each layer gets a dedicated engine so
  it never queues: second-to-last on the DVE (mul+max pair; a single
  scalar_tensor_tensor cannot read PSUM twice), last on the then-idle
  Scalar ACT.
- L3's two m-blocks accumulate k-interleaved so both psums complete right
  after h3's activation; the slower DVE mul+max pair gets the earlier
  psum (g0) while the Scalar ACT takes g1, and the classifier accumulates
  g1 first so its last matmul chases only the later-finishing act.
"""

from contextlib import ExitStack

import numpy as np

import concourse.bacc as bacc
import concourse.mybir as mybir
import concourse.tile as tile
from concourse.bass_utils import run_bass_kernel_spmd

F32 = mybir.dt.float32
F16 = mybir.dt.float16
P = 128
N_CORES = 8
B_FULL = 2048
B = B_FULL // N_CORES  # 256 rows per core
D0, D1, D2, D3 = 1024, 1024, 512, 256
K0, M0 = D0 // P, D1 // P  # 8, 8
K1, M1 = D1 // P, D2 // P  # 8, 4
K2, M2 = D2 // P, D3 // P  # 4, 2
KC = D3 // P  # 2

NEG_SLOPE = 0.2

# --- tunables (A/B'd on hardware) ---
# 19 x 213ns = 4.0us of sustained cold-clock activity: enough to flip the
# HAM clock gate (needs ~3.4us), ending just before the earliest plausible
# x-DMA receipt so the warmup never delays the first real matmul.
NWARM = 19

def _build(zero_bias: bool):
    nc = bacc.Bacc(
        "TRN2", target_bir_lowering=False, debug=False,
        enable_asserts=False, num_devices=1,
    )

    # Bulk inputs, split into chunks ordered by consumption time. Chunk k's
    # completion receipt must land before its first consumer; margins grow
    # toward the tail so SDMA hiccups can't stall the PE. DMA element size
    # (row bytes) stays >= 4KB — 2KB-row streams measured ~40% slower
    # aggregate. The W2+Wc tail (2KB rows) rides last with ~4us of slack.
    xw_d = nc.dram_tensor("xw", [P, K0 * B + 2 * K0 * P], F16,
                          kind="ExternalInput").ap()        # x^T + W0 m0-m1
    w0m23_d = nc.dram_tensor("w0m23", [P, 2 * K0 * P], F16,
                             kind="ExternalInput").ap()
    w0m45_d = nc.dram_tensor("w0m45", [P, 2 * K0 * P], F16,
                             kind="ExternalInput").ap()
    w0m67w1m0_d = nc.dram_tensor("w0m67w1m0", [P, 2 * K0 * P + K1 * P], F16,
                                 kind="ExternalInput").ap()
    w1m123_d = nc.dram_tensor("w1m123", [P, 3 * K1 * P], F16,
                              kind="ExternalInput").ap()
    w2wc_d = nc.dram_tensor("w2wc", [P, M2 * K2 * P + KC], F16,
                            kind="ExternalInput").ap()
    if not zero_bias:
        b0_d = nc.dram_tensor("b0", [D1], F32, kind="ExternalInput").ap()
        b1_d = nc.dram_tensor("b1", [D2], F32, kind="ExternalInput").ap()
        b2_d = nc.dram_tensor("b2", [D3], F32, kind="ExternalInput").ap()
        bc_d = nc.dram_tensor("bc", [1], F32, kind="ExternalInput").ap()
    out_d = nc.dram_tensor("out", [1, B], F32, kind="ExternalOutput").ap()

    with ExitStack() as ctx:
        tc = ctx.enter_context(tile.TileContext(nc))
        const = ctx.enter_context(tc.tile_pool(name="const", bufs=1))
        xt_p = ctx.enter_context(tc.tile_pool(name="xt", bufs=1))
        w_p = ctx.enter_context(tc.tile_pool(name="w", bufs=8))
        s_p = ctx.enter_context(tc.tile_pool(name="s", bufs=K1))
        h_p = ctx.enter_context(tc.tile_pool(name="h", bufs=K2))
        g_p = ctx.enter_context(tc.tile_pool(name="g", bufs=KC))
        tmp_p = ctx.enter_context(tc.tile_pool(name="tmp", bufs=4))
        out_p = ctx.enter_context(tc.tile_pool(name="outp", bufs=1))
        ps_p = ctx.enter_context(tc.tile_pool(name="ps", bufs=6, space="PSUM"))
        cls_ps = ctx.enter_context(tc.tile_pool(name="cls", bufs=1, space="PSUM"))
        warm_ps = ctx.enter_context(tc.tile_pool(name="warm", bufs=1,
                                                 space="PSUM"))

        # leaky-relu slope as a per-partition alpha vector for ACT Prelu
        alt = const.tile([P, 1], F32, tag="alt")
        nc.vector.memset(alt[:], NEG_SLOPE)

        # ---- PE warmup: HAM clock gate defaults to 4/8 (1.2GHz); ~3.4us of
        # sustained activity flips it to 8/8. Fill the input-DMA wait with a
        # zero matmul accumulation group sized to end ~when the first input
        # DMA's receipt fires; it must run CONTIGUOUSLY into the real work. ----
        wz = const.tile([P, B], F16, tag="wz")
        nc.vector.memset(wz[:], 0.0)
        pw = warm_ps.tile([P, B], F32)
        for i in range(NWARM):
            nc.tensor.matmul(pw[:], lhsT=wz[:, 0:P], rhs=wz[:],
                             start=(i == 0), stop=(i == NWARM - 1))

        # ---- DMA plan: ONE queue (sync HWDGE), chunks in consumption order.
        # HWDGE descriptor generation costs ~650ns per dma_start on the
        # issuing sequencer; 7 DMAs ~ 4.6us, which stays ahead of the
        # transfer stream. ----
        xw_t = xt_p.tile([P, K0 * B + 2 * K0 * P], F16, tag="xt", name="xw")
        nc.sync.dma_start(xw_t[:], xw_d)
        xt = [xw_t[:, k * B:(k + 1) * B] for k in range(K0)]

        chunks = []
        for nm, dram, cols in (
            ("w0m23", w0m23_d, 2 * K0 * P),
            ("w0m45", w0m45_d, 2 * K0 * P),
            ("w0m67w1m0", w0m67w1m0_d, 2 * K0 * P + K1 * P),
            ("w1m123", w1m123_d, 3 * K1 * P),
            ("w2wc", w2wc_d, M2 * K2 * P + KC),
        ):
            t = w_p.tile([P, cols], F16, tag="w", name=nm)
            nc.sync.dma_start(t[:], dram)
            chunks.append(t)
        w0m23_t, w0m45_t, w0m67w1m0_t, w1m123_t, w2wc_t = chunks
        wc = w2wc_t[:, M2 * K2 * P:M2 * K2 * P + KC]

        def w0_lhsT(m, k):
            if m < 2:
                off = K0 * B + (m * K0 + k) * P
                return xw_t[:, off:off + P]
            if m in (2, 3):
                off = ((m - 2) * K0 + k) * P
                return w0m23_t[:, off:off + P]
            if m in (4, 5):
                off = ((m - 4) * K0 + k) * P
                return w0m45_t[:, off:off + P]
            off = ((m - 6) * K0 + k) * P
            return w0m67w1m0_t[:, off:off + P]

        def w1_lhsT(m, k):
            if m == 0:
                off = 2 * K0 * P + k * P
                return w0m67w1m0_t[:, off:off + P]
            off = ((m - 1) * K1 + k) * P
            return w1m123_t[:, off:off + P]

        def w2_lhsT(m, k):
            return w2wc_t[:, (m * K2 + k) * P:(m * K2 + k + 1) * P]

        if not zero_bias:
            b0t = const.tile([P, M0], F32, tag="b0t")
            nc.scalar.dma_start(b0t[:], b0_d.rearrange("(c p) -> p c", p=P))
            b1t = const.tile([P, M1], F32, tag="b1t")
            nc.scalar.dma_start(b1t[:], b1_d.rearrange("(c p) -> p c", p=P))
            b2t = const.tile([P, M2], F32, tag="b2t")
            nc.scalar.dma_start(b2t[:], b2_d.rearrange("(c p) -> p c", p=P))
            bct = const.tile([1, 1], F32, tag="bct")
            nc.scalar.dma_start(bct[:], bc_d.rearrange("(a b) -> a b", a=1))
            b0t2 = const.tile([P, M0], F32, tag="b0t2")
            nc.vector.tensor_scalar_mul(b0t2[:], b0t[:], 2.0)

        PRELU = mybir.ActivationFunctionType.Prelu

        def matmul_group(ps, lhsT_fn, m, rhs_tiles, K):
            for k in range(K):
                nc.tensor.matmul(
                    ps[:], lhsT=lhsT_fn(m, k),
                    rhs=rhs_tiles[k],
                    start=(k == 0), stop=(k == K - 1),
                )

        def lrelu_dve(dst, ps, name):
            # DVE pair: max(0.2*v, v). A single scalar_tensor_tensor with
            # in0==in1==psum is rejected (one PSUM read port), so stage
            # 0.2*v in SBUF first.
            t = tmp_p.tile([P, B], F32, tag="l", name=name)
            nc.vector.tensor_scalar_mul(t[:], ps, NEG_SLOPE)
            nc.vector.tensor_max(dst, ps, t[:])

        # ---- layer 1: y[m] = sum_k W0[k,m].T @ xT[k]; s = lrelu(y)
        # (the degree-sum factor 12 is pre-folded into W1). The last
        # m-block's act runs on the DVE so L2 isn't queued behind the
        # Scalar-engine ACT pipeline. ----
        s_tiles = []
        for m in range(M0):
            ps = ps_p.tile([P, B], F32, tag="ps", name=f"ps1_{m}")
            matmul_group(ps, w0_lhsT, m, xt, K0)
            s = s_p.tile([P, B], F16, tag="s", name=f"s_{m}")
            if zero_bias:
                # second-to-last act on the DVE so the Scalar queue is empty
                # when the last m-block's psum completes -> its ACT starts
                # immediately and L2 isn't held up.
                if m == M0 - 2:
                    lrelu_dve(s[:], ps[:], f"sv_{m}")
                else:
                    nc.scalar.activation(s[:], ps[:], PRELU, alpha=alt[:])
            else:
                acc = tmp_p.tile([P, B], F32, tag="l", name=f"acc_{m}")
                first = True
                for scale, bias in ((1.0, b0t[:, m:m + 1]), (6.0, b0t2[:, m:m + 1]),
                                    (5.0, b0t[:, m:m + 1])):
                    l = tmp_p.tile([P, B], F32, tag="l", name=f"l_{m}")
                    nc.scalar.activation(l[:], ps[:], PRELU,
                                         scale=scale, bias=bias, alpha=alt[:])
                    if first:
                        nc.vector.tensor_copy(acc[:], l[:])
                        first = False
                    else:
                        nc.vector.tensor_add(acc[:], acc[:], l[:])
                nc.vector.tensor_copy(s[:], acc[:])
            s_tiles.append(s)

        # ---- layer 2: t[m] = sum_k W1[k,m].T @ s[k]; h = lrelu(t + b1) ----
        h_tiles = []
        for m in range(M1):
            ps = ps_p.tile([P, B], F32, tag="ps", name=f"ps2_{m}")
            matmul_group(ps, w1_lhsT, m, [t[:] for t in s_tiles], K1)
            h = h_p.tile([P, B], F16, tag="h", name=f"h_{m}")
            if zero_bias:
                if m == M1 - 2:
                    lrelu_dve(h[:], ps[:], f"hv_{m}")
                else:
                    nc.scalar.activation(h[:], ps[:], PRELU, alpha=alt[:])
            else:
                nc.scalar.activation(h[:], ps[:], PRELU,
                                     bias=b1t[:, m:m + 1], alpha=alt[:])
            h_tiles.append(h)

        # ---- layer 3, k-interleaved across the two m-blocks so both psums
        # complete right after h3's activation; g0 act on DVE and g1 act on
        # Scalar run concurrently; classifier matmuls chase them. ----
        ps30 = ps_p.tile([P, B], F32, tag="ps", name="ps3_0")
        ps31 = ps_p.tile([P, B], F32, tag="ps", name="ps3_1")
        for k in range(K2):
            nc.tensor.matmul(ps30[:], lhsT=w2_lhsT(0, k), rhs=h_tiles[k][:],
                             start=(k == 0), stop=(k == K2 - 1))
            nc.tensor.matmul(ps31[:], lhsT=w2_lhsT(1, k), rhs=h_tiles[k][:],
                             start=(k == 0), stop=(k == K2 - 1))
        g0 = g_p.tile([P, B], F16, tag="g", name="g_0")
        g1 = g_p.tile([P, B], F16, tag="g", name="g_1")
        if zero_bias:
            # the slower DVE mul+max pair gets the earlier-completing psum
            # (ps30); the single-instruction Scalar ACT takes ps31.
            lrelu_dve(g0[:], ps30[:], "gv")
            nc.scalar.activation(g1[:], ps31[:], PRELU, alpha=alt[:])
        else:
            nc.scalar.activation(g0[:], ps30[:], PRELU,
                                 bias=b2t[:, 0:1], alpha=alt[:])
            nc.scalar.activation(g1[:], ps31[:], PRELU,
                                 bias=b2t[:, 1:2], alpha=alt[:])

        # accumulation order is free: start with g1 (ready first via the
        # Scalar ACT) so the last matmul chases only g0's DVE pair.
        po = cls_ps.tile([1, B], F32)
        nc.tensor.matmul(po[:], lhsT=wc[:, 1:2], rhs=g1[:],
                         start=True, stop=False)
        nc.tensor.matmul(po[:], lhsT=wc[:, 0:1], rhs=g0[:],
                         start=False, stop=True)

        ob = out_p.tile([1, B], F32)
        if zero_bias:
            nc.vector.tensor_copy(ob[:], po[:])
        else:
            nc.vector.tensor_scalar_add(ob[:], po[:], bct[:, 0:1])
        nc.sync.dma_start(out_d, ob[:])

    nc.compile()
    return nc


_CACHE = {}


def _get_nc(zero_bias: bool):
    if zero_bias not in _CACHE:
        _CACHE[zero_bias] = _build(zero_bias)
    return _CACHE[zero_bias]


def _run(inputs, trace=False, **kw):
    def f32(a):
        return np.ascontiguousarray(np.asarray(a), dtype=np.float32)

    x = f32(inputs["x"])
    W0, b0 = f32(inputs["W0"]), f32(inputs["b0"])
    W1, b1 = f32(inputs["W1"]), f32(inputs["b1"])
    W2, b2 = f32(inputs["W2"]), f32(inputs["b2"])
    Wc, bc = f32(inputs["Wc"]), f32(inputs["bc"])
    zero_bias = not (b0.any() or b1.any() or b2.any() or bc.any())
    nc = _get_nc(zero_bias)

    # Host-side packing into exact SBUF tile layouts, fp16.
    def pack_w(W, K, M):
        return np.ascontiguousarray(
            W.astype(np.float16).reshape(K, P, M, P)
            .transpose(1, 2, 0, 3).reshape(P, M * K * P))

    w0p = pack_w(W0, K0, M0)
    # degree-sum factor of node 1's in-neighbourhood, folded into W1
    w1p = pack_w(W1 * 12.0 if zero_bias else W1, K1, M1)
    w2p = pack_w(W2, K2, M2)
    wcp = Wc.astype(np.float16)[:, 0].reshape(KC, P).T  # [128, 2]

    KP = K0 * P
    common = {
        "w0m23": np.ascontiguousarray(w0p[:, 2 * KP:4 * KP]),
        "w0m45": np.ascontiguousarray(w0p[:, 4 * KP:6 * KP]),
        "w0m67w1m0": np.ascontiguousarray(
            np.concatenate([w0p[:, 6 * KP:8 * KP], w1p[:, 0:K1 * P]], axis=1)),
        "w1m123": np.ascontiguousarray(w1p[:, K1 * P:4 * K1 * P]),
        "w2wc": np.ascontiguousarray(np.concatenate([w2p, wcp], axis=1)),
    }

    in_maps = []
    for i in range(N_CORES):
        xs = x[i * B:(i + 1) * B].astype(np.float16)  # [256, 1024]
        xtp = xs.T.reshape(K0, P, B).transpose(1, 0, 2).reshape(P, K0 * B)
        xwp = np.ascontiguousarray(
            np.concatenate([xtp, w0p[:, 0:2 * KP]], axis=1))
        m = {"xw": xwp, **common}
        if not zero_bias:
            m.update({"b0": b0, "b1": b1, "b2": b2, "bc": bc})
        in_maps.append(m)
    res = run_bass_kernel_spmd(nc, in_maps, list(range(N_CORES)),
                               trace=trace, **kw)
    out = np.empty((B_FULL, 1), dtype=np.float32)
    for i in range(N_CORES):
        out[i * B:(i + 1) * B, 0] = res.results[i]["out"][0]
    return out, res


def kernel(**inputs) -> np.ndarray:
    out, _ = _run(inputs)
    return out
